# revision 28
# baseline (speedup 1.0000x reference)
"""Dense transformer block (B=4,S=2048,E=1024,H=16) on 8 trn2 cores.

Sharding: 2 cores per batch sequence; core parity p takes rows p, p+2, ...
(stride-2 interleave) as its query rows -- this balances causal-attention
work exactly across cores.  Each core's x input is row-permuted to
[q rows (local order), other-parity rows] so every SBUF/DRAM offset in the
SPMD program is compile-time constant; causality is enforced with per-core
0/1 mask tensors (pure data).

v2 -- one long PE-dense software pipeline:
  A: per 512-row chunk: LN1 -> transpose -> K projection (Q proj for chunk0).
  Stream: attention runs pr-granular (one head pair at a time, 8 units per
     q-chunk).  Attention is ScalarE(exp)-bound, so independent PE work is
     issued between visits from an explicit slot schedule: V projection,
     Q proj chunk1, then out-proj + LN2 + FFN1 of q-chunk 0.
  D: FFN2(ch0); out-proj/LN2/FFN1/FFN2 of q-chunk 1 (PE-dense tail).
PSUM: psS (scores, 2x[128,1024]) 4 banks + psO (AV accum, 2x[128,512])
2 banks + shared projection accumulator (2x[128,512]) 2 banks = 8.
"""

import numpy as np

B, S, E, H, DH = 4, 2048, 1024, 16, 64
EPS = 1e-5
QR = S // 2          # q rows per core
CH = 512             # q-chunk (matmul free dim)
NCH = QR // CH       # 2 chunks
NKB = S // 128       # 16 key blocks
ET = E // 128        # 8 E tiles
NPR = H // 2         # 8 head pairs
FE = 4 * E           # ffn hidden
NS4 = FE // 128      # 32 ffn hidden slices
SC = 1.0 / np.sqrt(DH)

_PROG = None


def _visits(ch):
    """per q-chunk: list of (key_block, qlo or None) in accumulation order"""
    if ch == 0:
        return [(kb, 128 * (kb % 8)) for kb in (0, 1, 2, 3, 8, 9, 10, 11)]
    full = [(kb, None) for kb in (0, 1, 2, 3, 8, 9, 10, 11)]
    diag = [(kb, 128 * ((kb - 4) if kb < 8 else (kb - 12)))
            for kb in (4, 5, 6, 7, 12, 13, 14, 15)]
    return full + diag


def _build():
    import concourse.bacc as bacc
    import concourse.tile as tile
    from concourse import mybir
    from concourse.masks import make_identity

    F32 = mybir.dt.float32
    F32R = mybir.dt.float32r
    BF16 = mybir.dt.bfloat16
    AF = mybir.ActivationFunctionType

    nc = bacc.Bacc("TRN2", target_bir_lowering=False, debug=False, num_devices=8)

    xin = nc.dram_tensor("xin", [S, E], F32, kind="ExternalInput").ap()
    masks = nc.dram_tensor("masks", [2, 128, 256], BF16, kind="ExternalInput").ap()
    sel = nc.dram_tensor("sel", [128, 128], BF16, kind="ExternalInput").ap()
    # pre-transposed host-side: [ei, eo, h, d]
    wq = nc.dram_tensor("wq", [128, ET, H, DH], BF16, kind="ExternalInput").ap()
    wk = nc.dram_tensor("wk", [128, ET, H, DH], BF16, kind="ExternalInput").ap()
    wv = nc.dram_tensor("wv", [128, ET, H, DH], BF16, kind="ExternalInput").ap()
    wo = nc.dram_tensor("wo", [E, E], BF16, kind="ExternalInput").ap()
    bo = nc.dram_tensor("bo", [E], BF16, kind="ExternalInput").ap()
    ln1g = nc.dram_tensor("ln1g", [E], F32, kind="ExternalInput").ap()
    ln1b = nc.dram_tensor("ln1b", [E], F32, kind="ExternalInput").ap()
    ln2g = nc.dram_tensor("ln2g", [E], F32, kind="ExternalInput").ap()
    ln2b = nc.dram_tensor("ln2b", [E], F32, kind="ExternalInput").ap()
    # pre-transposed host-side: [ei, s4, eo, fi]
    w1 = nc.dram_tensor("w1", [128, NS4, ET, 128], BF16, kind="ExternalInput").ap()
    b1 = nc.dram_tensor("b1", [FE], F32, kind="ExternalInput").ap()
    w2 = nc.dram_tensor("w2", [FE, E], BF16, kind="ExternalInput").ap()
    b2 = nc.dram_tensor("b2", [E], BF16, kind="ExternalInput").ap()
    out = nc.dram_tensor("out", [QR, E], F32, kind="ExternalOutput").ap()

    with tile.TileContext(nc, pool_alloc_mode="queue") as tc:
        consts = tc.alloc_tile_pool(name="consts", bufs=1)
        small = tc.alloc_tile_pool(name="small", bufs=6)

        ident = consts.tile([128, 128], F32)
        make_identity(nc, ident)
        identr = consts.tile([128, 128], F32R, tag="identr")
        nc.vector.tensor_copy(identr, ident)
        onesb = consts.tile([128, 256], BF16, tag="onesb")
        nc.vector.memset(onesb, 1.0)
        epst = consts.tile([128, 1], F32)
        nc.vector.memset(epst, EPS)
        sel_sb = consts.tile([128, 128], BF16, tag="sel")
        nc.sync.dma_start(sel_sb, sel)
        ln1g_sb = consts.tile([128, ET], F32, tag="lnp1")
        nc.sync.dma_start(ln1g_sb, ln1g.rearrange("(eo ei) -> ei eo", ei=128))
        ln1b_sb = consts.tile([128, ET], F32, tag="lnp2")
        nc.sync.dma_start(ln1b_sb, ln1b.rearrange("(eo ei) -> ei eo", ei=128))
        ln2g_sb = consts.tile([128, ET], F32, tag="lnp3")
        nc.sync.dma_start(ln2g_sb, ln2g.rearrange("(eo ei) -> ei eo", ei=128))
        ln2b_sb = consts.tile([128, ET], F32, tag="lnp4")
        nc.sync.dma_start(ln2b_sb, ln2b.rearrange("(eo ei) -> ei eo", ei=128))
        b1_sb = consts.tile([128, NS4], F32, tag="b1")
        nc.sync.dma_start(b1_sb, b1.rearrange("(so si) -> si so", si=128))
        wedges = []
        for w in range(2):
            mt = consts.tile([128, 256], BF16, tag=f"mask{w}", name=f"wedge{w}")
            nc.sync.dma_start(mt, masks[w])
            wedges.append(mt)
        rsums = consts.tile([128, 512], F32, tag="rsums")
        nc.vector.memset(rsums, 1.0)  # rows off {0,64} stay 1.0 (benign)

        def layernorm_rows(x_tiles, n_tiles, nrow_tiles):
            """natural-layout LN stats+center+scale for a list of row tiles"""
            for j in range(nrow_tiles):
                xt = x_tiles[j]
                st = small.tile([128, 2, 6], F32, tag="bnst")
                xr = xt.rearrange("p (a b) -> p a b", a=2)
                for sg in range(2):
                    nc.vector.bn_stats(st[:, sg, :], xr[:, sg, :])
                mv = small.tile([128, 2], F32, tag="bnmv")
                nc.vector.bn_aggr(mv, st)
                rstd = small.tile([128, 1], F32, tag="rstd")
                nc.scalar.activation(rstd, mv[:, 1:2], AF.Sqrt, bias=epst)
                nc.vector.reciprocal(rstd, rstd)
                nc.vector.tensor_scalar(
                    n_tiles[j], xt, mv[:, 0:1], rstd,
                    mybir.AluOpType.subtract, mybir.AluOpType.mult,
                )

        # ---------------- long-lived left-stack pools ----------------
        nrmp = tc.alloc_tile_pool(name="nrm", bufs=1)
        w1s = tc.alloc_tile_pool(name="w1s", bufs=2)
        xn2 = tc.alloc_tile_pool(name="xn2", bufs=2)

        # right stack (LIFO top-down): attention state below, early-released
        # weight/Y1T pools above so they pop first (wk after A; wq/wv/y1t
        # mid-stream; pt/vsb/att after the attention stream).
        att_pool = tc.alloc_tile_pool(name="att", bufs=NPR, side="right")
        QT = [att_pool.tile([128, QR], BF16, tag="qt", name=f"QT{i}") for i in range(NPR)]
        KT = [att_pool.tile([128, S], BF16, tag="kt", name=f"KT{i}") for i in range(NPR)]
        vsb_pool = tc.alloc_tile_pool(name="vsb", bufs=1, side="right")
        VSB = vsb_pool.tile([128, NKB, H, DH + 1], BF16, tag="vsb", name="VSB")
        nc.vector.memset(VSB[:, :, :, DH], 1.0)
        ptp = tc.alloc_tile_pool(name="pt", bufs=2, side="right")
        y1t_pool = tc.alloc_tile_pool(name="y1t", bufs=ET, side="right")
        Y1T = [y1t_pool.tile([128, S], BF16, tag="y1t", name=f"Y1T{i}") for i in range(ET)]
        wvp = tc.alloc_tile_pool(name="wvp", bufs=1, side="right")
        wv_all = wvp.tile([128, ET, H, DH], BF16, tag="wva", name="wv_all")
        wqp = tc.alloc_tile_pool(name="wqp", bufs=1, side="right")
        wq_all = wqp.tile([128, ET, H, DH], BF16, tag="wqa", name="wq_all")
        wkp = tc.alloc_tile_pool(name="wkp", bufs=1, side="right")
        wk_all = wkp.tile([128, ET, H, DH], BF16, tag="wka", name="wk_all")

        # =============== Phase A: LN1 -> Y1T; K proj; Q proj ch0 ===============
        with (
            tc.tile_pool(name="xtn", bufs=2) as xtn,
            tc.tile_pool(name="psT", bufs=2, space="PSUM") as psT,
            tc.tile_pool(name="psP1", bufs=2, space="PSUM") as psP1a,
        ):
            for rc in range(4):
                for g in range(2):
                    xts, n1s = [], []
                    for j in range(2):
                        ri = rc * 4 + g * 2 + j
                        xt = xtn.tile([128, E], F32, tag="xt", name=f"xt{j}")
                        nc.sync.dma_start(xt, xin[ri * 128:(ri + 1) * 128, :])
                        xts.append(xt)
                        n1s.append(xtn.tile([128, E], F32R, tag="n1", name=f"n1s{j}"))
                    # weight DMAs in 1MB halves interleaved behind the
                    # x-row loads so each chunk's LN input never queues
                    # behind a full 2MB transfer
                    wsched = {(0, 0): (wk_all, wk, 0), (0, 1): (wk_all, wk, 1),
                              (1, 0): (wq_all, wq, 0), (1, 1): (wq_all, wq, 1),
                              (2, 0): (wv_all, wv, 0), (2, 1): (wv_all, wv, 1)}
                    if (rc, g) in wsched:
                        dst, srcw, hf = wsched[(rc, g)]
                        nc.sync.dma_start(dst[:, 4 * hf:4 * (hf + 1)],
                                          srcw[:, 4 * hf:4 * (hf + 1)])
                    layernorm_rows(xts, n1s, 2)
                    for e in range(ET):
                        ps = psT.tile([128, 256], F32R, tag="pst")
                        for j in range(2):
                            nc.tensor.transpose(
                                ps[:, j * 128:(j + 1) * 128],
                                n1s[j][:, e * 128:(e + 1) * 128], identr)
                        dst = Y1T[e][:, (rc * 2 + g) * 256:(rc * 2 + g + 1) * 256]
                        if e % 2:
                            nc.scalar.activation(
                                dst, ps.bitcast(F32), AF.Identity,
                                bias=ln1b_sb[:, e:e + 1],
                                scale=ln1g_sb[:, e:e + 1])
                        else:
                            nc.vector.tensor_scalar(
                                dst, ps.bitcast(F32),
                                ln1g_sb[:, e:e + 1], ln1b_sb[:, e:e + 1],
                                mybir.AluOpType.mult, mybir.AluOpType.add)
                # K projection for this 512-key chunk, all head pairs
                for pr in range(NPR):
                    ps = psP1a.tile([128, 512], F32, tag="proj")
                    for e in range(ET):
                        nc.tensor.matmul(
                            ps, wk_all[:, e, 2 * pr:2 * pr + 2, :],
                            Y1T[e][:, rc * 512:(rc + 1) * 512],
                            start=(e == 0), stop=(e == ET - 1))
                    nc.vector.tensor_copy(KT[pr][:, rc * 512:(rc + 1) * 512], ps)
                if rc in (1, 2):
                    # Q proj (chunk-0 cols, still resident) deferred so its
                    # weights can stream during rc1; split across rc1/rc2
                    for pr in range(4 * (rc - 1), 4 * rc):
                        ps = psP1a.tile([128, 512], F32, tag="proj")
                        for e in range(ET):
                            nc.tensor.matmul(
                                ps, wq_all[:, e, 2 * pr:2 * pr + 2, :],
                                Y1T[e][:, 0:512],
                                start=(e == 0), stop=(e == ET - 1))
                        nc.vector.tensor_copy(QT[pr][:, 0:512], ps)
        wkp.release()

        oac_pool = tc.alloc_tile_pool(name="oac", bufs=NPR)
        OACC = [oac_pool.tile([128, QR], BF16, tag="oacc", name=f"OACC{i}")
                for i in range(NPR)]
        x2_pool = tc.alloc_tile_pool(name="x2", bufs=1)
        shared = tc.alloc_tile_pool(name="shr", bufs=2, space="PSUM")
        psS = tc.alloc_tile_pool(name="psS", bufs=2, space="PSUM")
        psO = tc.alloc_tile_pool(name="psO", bufs=2, space="PSUM")

        pools = {}    # mid-stream allocated pools (x2/y2t/h1/w1s/xn2/wop)
        wo_sb_box = {}
        X2 = {}       # chunk -> [128, 4, E] f32 tile (lazy)
        Y2T = {}      # chunk -> [e][128, CH] bf16 (lazy)
        h1t = {0: {}, 1: {}}   # chunk -> s4 -> [128, CH] bf16 (lazy)

        def issue_avs(av):
            ops, pr, kb, q0, pt, first, last = av
            N = CH - q0
            for hh in range(2):
                nc.tensor.matmul(
                    ops[hh][0:DH + 1, q0:CH],
                    VSB[:, kb, 2 * pr + hh, :],
                    pt[:, hh * 512:hh * 512 + N],
                    start=first, stop=last, skip_group_check=True)

        def issue_epilogue(ep):
            ops, pr, ch = ep
            for hh in range(2):
                eng = nc.scalar.copy if hh else nc.vector.tensor_copy
                eng(rsums[64 * hh:64 * hh + 1, :], ops[hh][DH:DH + 1, :])
            for hh in range(2):
                nc.vector.tensor_copy(
                    OACC[pr][hh * 64:(hh + 1) * 64, ch * CH:(ch + 1) * CH],
                    ops[hh][0:DH, :])
            rcp = nrmp.tile([128, 512], F32, tag="rcp")
            nc.vector.reciprocal(rcp, rsums)
            return (rcp, pr, ch)

        def issue_norm(nm, shared_bc=False):
            rcp, pr, ch = nm
            rcpb = nrmp.tile([128, 512], BF16, tag="rcpb")
            nc.vector.tensor_copy(rcpb, rcp)
            if shared_bc:
                bc = shared.tile([128, 512], F32, tag="proj")
                bcv = bc
            else:
                bc = psS.tile([128, 1024], F32, tag="sc")
                bcv = bc[:, 0:512]
            nc.tensor.matmul(
                bcv, sel_sb[0:65, :], rcpb[0:65, :],
                start=True, stop=True)
            bcs = nrmp.tile([128, 512], BF16, tag="bcs")
            nc.vector.tensor_copy(bcs, bcv)
            nc.vector.tensor_mul(
                OACC[pr][:, ch * CH:(ch + 1) * CH],
                OACC[pr][:, ch * CH:(ch + 1) * CH], bcs)

        # ---- filler blocks ----
        def v_block(kb, half):
            def go():
                ps = shared.tile([128, 512], F32, tag="proj")
                for e in range(ET):
                    nc.tensor.matmul(
                        ps, Y1T[e][:, kb * 128:(kb + 1) * 128],
                        wv_all[:, e, 8 * half:8 * half + 8, :],
                        start=(e == 0), stop=(e == ET - 1))
                nc.vector.tensor_copy(
                    VSB[:, kb, 8 * half:8 * half + 8, 0:DH],
                    ps.rearrange("p (h d) -> p h d", h=8))
            return go

        def q1_block(pr):
            def go():
                ps = shared.tile([128, 512], F32, tag="proj")
                for e in range(ET):
                    nc.tensor.matmul(
                        ps, wq_all[:, e, 2 * pr:2 * pr + 2, :],
                        Y1T[e][:, 512:1024],
                        start=(e == 0), stop=(e == ET - 1))
                nc.vector.tensor_copy(QT[pr][:, 512:1024], ps)
            return go

        def release_block():
            def go():
                wqp.release()
                wvp.release()
                y1t_pool.release()
            return go

        def alloc_pools_block():
            def go():
                pools["y2t"] = tc.alloc_tile_pool(name="y2t", bufs=ET)
                pools["h1"] = tc.alloc_tile_pool(name="h1", bufs=NS4)
            return go

        def wo_load_block():
            def go():
                wop = tc.alloc_tile_pool(name="wop", bufs=1)
                pools["wop"] = wop
                wo_sb = wop.tile([128, ET, E], BF16, tag="wo")
                nc.sync.dma_start(
                    wo_sb, wo.rearrange("(po pi) o -> pi po o", pi=128))
                wo_sb_box[0] = wo_sb
                bo_sb = wop.tile([1, E], BF16, tag="bo")
                nc.sync.dma_start(bo_sb, bo[None, :])
                wo_sb_box["bo"] = bo_sb
                b2_sb = wop.tile([1, E], BF16, tag="b2")
                nc.sync.dma_start(b2_sb, b2[None, :])
                wo_sb_box["b2"] = b2_sb
            return go

        def x2_init_block(c, j):
            def go():
                if c not in X2:
                    X2[c] = x2_pool.tile([128, 4, E], F32, tag="x2",
                                         name=f"X2_{c}")
                qt = c * 4 + j
                nc.sync.dma_start(
                    X2[c][:, j, :], xin[qt * 128:(qt + 1) * 128, :])
            return go

        def p3_block(c, qt):
            def go():
                wo_sb = wo_sb_box[0]
                for eh in range(2):
                    ps = shared.tile([128, 512], F32, tag="proj")
                    for pr in range(NPR):
                        nc.tensor.matmul(
                            ps, OACC[pr][:, qt * 128:(qt + 1) * 128],
                            wo_sb[:, pr, eh * 512:(eh + 1) * 512],
                            start=(pr == 0), stop=False)
                    nc.tensor.matmul(
                        ps, onesb[0:1, 0:128],
                        wo_sb_box["bo"][0:1, eh * 512:(eh + 1) * 512],
                        start=False, stop=True)
                    nc.vector.tensor_add(
                        X2[c][:, qt % 4, eh * 512:(eh + 1) * 512],
                        X2[c][:, qt % 4, eh * 512:(eh + 1) * 512], ps)
            return go

        def ln2_block(c, j2):
            def go():
                if c not in Y2T:
                    Y2T[c] = [pools["y2t"].tile([128, CH], BF16, tag="y2t",
                                            name=f"Y2T{c}_{i}")
                              for i in range(ET)]
                x2s = [X2[c][:, j2 * 2 + j, :] for j in range(2)]
                n2s = [xn2.tile([128, E], F32R, tag="n2", name=f"n2s{j}")
                       for j in range(2)]
                layernorm_rows(x2s, n2s, 2)
                for e in range(ET):
                    ps = shared.tile([128, 512], F32R, tag="proj")
                    for j in range(2):
                        nc.tensor.transpose(
                            ps[:, j * 128:(j + 1) * 128],
                            n2s[j][:, e * 128:(e + 1) * 128], identr)
                    if e % 2:
                        nc.scalar.activation(
                            Y2T[c][e][:, j2 * 256:(j2 + 1) * 256],
                            ps[:, 0:256].bitcast(F32), AF.Identity,
                            bias=ln2b_sb[:, e:e + 1],
                            scale=ln2g_sb[:, e:e + 1])
                    else:
                        nc.vector.tensor_scalar(
                            Y2T[c][e][:, j2 * 256:(j2 + 1) * 256],
                            ps[:, 0:256].bitcast(F32),
                            ln2g_sb[:, e:e + 1], ln2b_sb[:, e:e + 1],
                            mybir.AluOpType.mult, mybir.AluOpType.add)
            return go

        def ffn1_block(c, g, pool=None):
            # processes s4 pair (2g, 2g+1) with one double-size weight DMA
            def go():
                w1_sb = w1s.tile([128, 2, ET, 128], BF16, tag="w1")
                nc.sync.dma_start(w1_sb, w1[:, 2 * g:2 * g + 2])
                for k in range(2):
                    s4 = 2 * g + k
                    if pool is not None:
                        ps = pool.tile([128, 512], F32, tag="f2")
                    else:
                        ps = shared.tile([128, 512], F32, tag="proj")
                    for e in range(ET):
                        nc.tensor.matmul(
                            ps, w1_sb[:, k, e, :], Y2T[c][e],
                            start=(e == 0), stop=(e == ET - 1))
                    ht = pools["h1"].tile([128, CH], BF16, tag="h1",
                                          name=f"h1_{c}_{s4}")
                    h1t[c][s4] = ht
                    # bias + relu on DVE (ScalarE stays free for softmax exp)
                    nc.vector.tensor_scalar(
                        ht, ps, b1_sb[:, s4:s4 + 1], 0.0,
                        mybir.AluOpType.add, mybir.AluOpType.max)
            return go

        # ---- filler schedule: slot -> list of blocks ----
        ch0_kbs = (0, 1, 2, 3, 8, 9, 10, 11)
        ch1_kbs = (4, 5, 6, 7, 12, 13, 14, 15)
        sched = {}

        def at(slot, blk):
            sched.setdefault(slot, []).append(blk)

        for i, kb in enumerate(ch0_kbs):
            at(i, v_block(kb, 0))              # c0pr0: JIT for its AVs
            at(8 + i, v_block(kb, 1))          # before pr4 (slot 32)
        for pr in range(NPR):                  # Q1 before ch1 (slot 64)
            at(16 + 4 * pr, q1_block(pr))
        for i, kb in enumerate(ch1_kbs):
            at(18 + 4 * i, v_block(kb, 0))     # before c1pr0 diag AVs (~73)
            at(48 + 3 * i, v_block(kb, 1))     # before c1pr4 (slot 128)
        at(70, release_block())                # after last V block @ 69
        at(71, alloc_pools_block())
        at(72, wo_load_block())
        for qt in range(4):
            at(73 + qt, x2_init_block(0, qt))
            at(77 + 2 * qt, p3_block(0, qt))   # after norm(c0pr7) @ slot 69
        at(85, ln2_block(0, 0))
        at(87, ln2_block(0, 1))
        for g in range(NS4 // 2):              # spread over slots 89..190
            at(89 + (g * 101) // 16, ffn1_block(0, g))

        # ---- the attention stream ----
        units = [(0, pr) for pr in range(NPR)] + [(1, pr) for pr in range(NPR)]
        slot = 0
        pend_ep = None
        norm_q = []
        for ch, pr in units:
            visits = _visits(ch)
            ops = [psO.tile([128, 512], F32, tag="ot", name=f"ot{hh}")
                   for hh in range(2)]
            pend_av = None
            nv = len(visits)
            for vi, (kb, qlo) in enumerate(visits):
                q0 = 0 if qlo is None else qlo
                N = CH - q0
                kcol = kb * 128
                wm = wedges[0 if kb < 8 else 1]
                pss = psS.tile([128, 1024], F32, tag="sc")
                for hh in range(2):
                    nc.tensor.matmul(
                        pss[:, hh * 512:hh * 512 + N],
                        KT[pr][hh * 64:(hh + 1) * 64, kcol:kcol + 128],
                        QT[pr][hh * 64:(hh + 1) * 64,
                               ch * CH + q0:(ch + 1) * CH],
                        start=True, stop=True)
                pt = ptp.tile([128, 1024], BF16, tag="pt")
                if N == 512:
                    nc.scalar.activation(pt, pss, AF.Exp, scale=SC)
                else:
                    pt3 = pt.rearrange("p (h c) -> p h c", h=2)
                    ps3 = pss.rearrange("p (h c) -> p h c", h=2)
                    nc.scalar.activation(pt3[:, :, 0:N], ps3[:, :, 0:N],
                                         AF.Exp, scale=SC)
                if qlo is not None:
                    pt3 = pt.rearrange("p (h c) -> p h c", h=2)
                    nc.vector.tensor_mul(
                        pt3[:, :, 0:128], pt3[:, :, 0:128],
                        wm.rearrange("p (a b) -> p a b", a=2))
                if vi == 1 and pend_ep is not None:
                    norm_q.append(issue_epilogue(pend_ep))
                    pend_ep = None
                if vi == 5 and norm_q:
                    issue_norm(norm_q.pop(0))
                if pend_av is not None:
                    issue_avs(pend_av)
                pend_av = (ops, pr, kb, q0, pt, vi == 0, vi == nv - 1)
                for blk in sched.get(slot, []):
                    blk()
                slot += 1
            issue_avs(pend_av)
            pend_ep = (ops, pr, ch)
        ptp.release()
        vsb_pool.release()
        att_pool.release()
        norm_q.append(issue_epilogue(pend_ep))
        psO.release()
        psS.release()
        for nm in norm_q:
            issue_norm(nm, shared_bc=True)

        # =============== Phase D: FFN2(0) | P3/LN2(1) interleaved ===========
        # w2 fully resident (both halves); X2(0) copied aside so the X2
        # buffer can rotate to chunk 1 while FFN2(0) is still consuming it.
        psF = tc.alloc_tile_pool(name="psF", bufs=4, space="PSUM")
        w2s = tc.alloc_tile_pool(name="w2s", bufs=1)
        otp = tc.alloc_tile_pool(name="otp", bufs=2)
        xsp = tc.alloc_tile_pool(name="xsp", bufs=1)
        w2all = w2s.tile([128, NS4, E], BF16, tag="w2a", name="w2all")
        w2r = w2.rearrange("(so si) e -> si so e", si=128)
        nc.sync.dma_start(w2all[:, 0:2, :], w2r[:, 0:2, :])
        nc.sync.dma_start(w2all[:, 2:4, :], w2r[:, 2:4, :])
        X2S = xsp.tile([128, 4, E], F32, tag="x2s", name="X2S")
        nc.vector.tensor_copy(X2S, X2[0])
        x2_init_block(1, 0)()
        for g in range(2, 16):
            nc.sync.dma_start(w2all[:, 2 * g:2 * (g + 1), :],
                              w2r[:, 2 * g:2 * (g + 1), :])
        for j in range(1, 4):
            x2_init_block(1, j)()

        def ffn2_group(c, eh, j, xsrc):
            ps = psF.tile([128, 512], F32, tag="f2")
            for s4 in range(NS4):
                nc.tensor.matmul(
                    ps, h1t[c][s4][:, j * 128:(j + 1) * 128],
                    w2all[:, s4, eh * 512:(eh + 1) * 512],
                    start=(s4 == 0), stop=False)
            nc.tensor.matmul(
                ps, onesb[0:1, 0:128],
                wo_sb_box["b2"][0:1, eh * 512:(eh + 1) * 512],
                start=False, stop=True)
            qt = c * 4 + j
            ot = otp.tile([128, 512], F32, tag="stg")
            nc.vector.tensor_add(ot, ps,
                                 xsrc[:, j, eh * 512:(eh + 1) * 512])
            nc.sync.dma_start(
                out[qt * 128:(qt + 1) * 128, eh * 512:(eh + 1) * 512], ot)

        d_fill = [(eh, j) for eh in range(2) for j in range(4)]
        d_work = ([lambda qt=qt: p3_block(1, qt)() for qt in range(4, 8)]
                  + [lambda: ln2_block(1, 0)(), lambda: ln2_block(1, 1)()])
        for i in range(8):
            eh, j = d_fill[i]
            ffn2_group(0, eh, j, X2S)
            if i < len(d_work):
                d_work[i]()
        for g in range(NS4 // 2):
            ffn1_block(1, g, psF)()
        for eh in range(2):
            for j in range(4):
                ffn2_group(1, eh, j, X2[1])

        xsp.release()
        otp.release()
        w2s.release()
        psF.release()
        shared.release()
        pools["wop"].release()
        pools["h1"].release()
        pools["y2t"].release()
        x2_pool.release()
        oac_pool.release()
        xn2.release()
        w1s.release()
        nrmp.release()
        small.release()
        consts.release()

    nc.compile()
    return nc


def _prep_inputs(inputs):
    import ml_dtypes
    BF = ml_dtypes.bfloat16
    x = np.ascontiguousarray(inputs["x"], dtype=np.float32)
    selm = np.zeros((128, 128), np.float32)
    selm[0, 0:64] = 1.0
    selm[64, 64:128] = 1.0
    shared = {
        "sel": selm.astype(BF),
        "wq": np.ascontiguousarray(
            np.asarray(inputs["Wq"]).reshape(H, ET, 128, DH)
            .transpose(2, 1, 0, 3)).astype(BF),
        "wk": np.ascontiguousarray(
            np.asarray(inputs["Wk"]).reshape(H, ET, 128, DH)
            .transpose(2, 1, 0, 3)).astype(BF),
        "wv": np.ascontiguousarray(
            np.asarray(inputs["Wv"]).reshape(H, ET, 128, DH)
            .transpose(2, 1, 0, 3)).astype(BF),
        "wo": np.ascontiguousarray(inputs["Wo"]).astype(BF),
        "bo": np.ascontiguousarray(inputs["bo"]).astype(BF),
        "ln1g": np.ascontiguousarray(inputs["ln1_g"], np.float32),
        "ln1b": np.ascontiguousarray(inputs["ln1_b"], np.float32),
        "ln2g": np.ascontiguousarray(inputs["ln2_g"], np.float32),
        "ln2b": np.ascontiguousarray(inputs["ln2_b"], np.float32),
        "w1": np.ascontiguousarray(
            np.asarray(inputs["W1"]).reshape(ET, 128, NS4, 128)
            .transpose(1, 2, 0, 3)).astype(BF),
        "b1": np.ascontiguousarray(inputs["b1"], np.float32),
        "w2": np.ascontiguousarray(inputs["W2"]).astype(BF),
        "b2": np.ascontiguousarray(inputs["b2"]).astype(BF),
    }
    in_maps = []
    for c in range(8):
        b, p = c // 2, c % 2
        perm = np.concatenate([np.arange(p, S, 2), np.arange(1 - p, S, 2)])
        kk = np.arange(128)[:, None]
        qq = np.arange(128)[None, :]
        m = np.zeros((2, 128, 128), np.float32)
        m[0] = (qq >= kk).astype(np.float32)          # own-parity blocks
        if p == 0:
            m[1] = (qq > kk).astype(np.float32)       # other-parity, even core
        else:
            m[1] = (qq >= kk).astype(np.float32)      # other-parity, odd core
        m2 = np.concatenate([m, m], axis=2)           # [2,128,256]: wedge doubled
        im = dict(shared)
        im["xin"] = np.ascontiguousarray(x[b][perm])
        im["masks"] = m2.astype(BF)
        in_maps.append(im)
    return in_maps


def _get_prog():
    global _PROG
    if _PROG is None:
        _PROG = _build()
    return _PROG


def run(inputs, trace=False):
    from concourse.bass_utils import run_bass_kernel_spmd

    nc = _get_prog()
    in_maps = _prep_inputs(inputs)
    kw = {}
    if trace:
        import sys, types
        try:
            from antenv.axon_hooks import get_axon_ntff_profile_hook  # noqa
        except ImportError:
            from trn_agent_boot.trn_boot import _ntff_profile_via_ctypes
            hook = _ntff_profile_via_ctypes("/opt/axon/libaxon_pjrt.so")
            mod = types.ModuleType("antenv.axon_hooks")
            mod.get_axon_ntff_profile_hook = lambda: hook
            sys.modules["antenv.axon_hooks"] = mod
        kw["trace"] = True
    res = run_bass_kernel_spmd(nc, in_maps, core_ids=list(range(8)), **kw)
    x = inputs["x"]
    outp = np.empty((B, S, E), np.float32)
    for c in range(8):
        b, p = c // 2, c % 2
        outp[b, p::2, :] = res.results[c]["out"]
    return outp, res


def kernel(**inputs):
    outp, _ = run(inputs)
    return outp


# revision 30
# speedup vs baseline: 1.0035x; 1.0035x over previous
"""Dense transformer block (B=4,S=2048,E=1024,H=16) on 8 trn2 cores.

Sharding: 2 cores per batch sequence; core parity p takes rows p, p+2, ...
(stride-2 interleave) as its query rows -- this balances causal-attention
work exactly across cores.  Each core's x input is row-permuted to
[q rows (local order), other-parity rows] so every SBUF/DRAM offset in the
SPMD program is compile-time constant; causality is enforced with per-core
0/1 mask tensors (pure data).

v2 -- one long PE-dense software pipeline:
  A: per 512-row chunk: LN1 -> transpose -> K projection (Q proj for chunk0).
  Stream: attention runs pr-granular (one head pair at a time, 8 units per
     q-chunk).  Attention is ScalarE(exp)-bound, so independent PE work is
     issued between visits from an explicit slot schedule: V projection,
     Q proj chunk1, then out-proj + LN2 + FFN1 of q-chunk 0.
  D: FFN2(ch0); out-proj/LN2/FFN1/FFN2 of q-chunk 1 (PE-dense tail).
PSUM: psS (scores, 2x[128,1024]) 4 banks + psO (AV accum, 2x[128,512])
2 banks + shared projection accumulator (2x[128,512]) 2 banks = 8.
"""

import numpy as np

B, S, E, H, DH = 4, 2048, 1024, 16, 64
EPS = 1e-5
QR = S // 2          # q rows per core
CH = 512             # q-chunk (matmul free dim)
NCH = QR // CH       # 2 chunks
NKB = S // 128       # 16 key blocks
ET = E // 128        # 8 E tiles
NPR = H // 2         # 8 head pairs
FE = 4 * E           # ffn hidden
NS4 = FE // 128      # 32 ffn hidden slices
SC = 1.0 / np.sqrt(DH)

_PROG = None


def _visits(ch):
    """per q-chunk: list of (key_block, qlo or None) in accumulation order"""
    if ch == 0:
        return [(kb, 128 * (kb % 8)) for kb in (0, 1, 2, 3, 8, 9, 10, 11)]
    full = [(kb, None) for kb in (0, 1, 2, 3, 8, 9, 10, 11)]
    diag = [(kb, 128 * ((kb - 4) if kb < 8 else (kb - 12)))
            for kb in (4, 5, 6, 7, 12, 13, 14, 15)]
    return full + diag


def _build():
    import concourse.bacc as bacc
    import concourse.tile as tile
    from concourse import mybir
    from concourse.masks import make_identity

    F32 = mybir.dt.float32
    F32R = mybir.dt.float32r
    BF16 = mybir.dt.bfloat16
    AF = mybir.ActivationFunctionType

    nc = bacc.Bacc("TRN2", target_bir_lowering=False, debug=False, num_devices=8)

    xin = nc.dram_tensor("xin", [S, E], F32, kind="ExternalInput").ap()
    masks = nc.dram_tensor("masks", [2, 128, 256], BF16, kind="ExternalInput").ap()
    sel = nc.dram_tensor("sel", [128, 128], BF16, kind="ExternalInput").ap()
    # pre-transposed host-side: [ei, eo, h, d]
    wq = nc.dram_tensor("wq", [128, ET, H, DH], BF16, kind="ExternalInput").ap()
    wk = nc.dram_tensor("wk", [128, ET, H, DH], BF16, kind="ExternalInput").ap()
    wv = nc.dram_tensor("wv", [128, ET, H, DH], BF16, kind="ExternalInput").ap()
    wo = nc.dram_tensor("wo", [E, E], BF16, kind="ExternalInput").ap()
    bo = nc.dram_tensor("bo", [E], BF16, kind="ExternalInput").ap()
    ln1g = nc.dram_tensor("ln1g", [E], F32, kind="ExternalInput").ap()
    ln1b = nc.dram_tensor("ln1b", [E], F32, kind="ExternalInput").ap()
    ln2g = nc.dram_tensor("ln2g", [E], F32, kind="ExternalInput").ap()
    ln2b = nc.dram_tensor("ln2b", [E], F32, kind="ExternalInput").ap()
    # pre-transposed host-side: [ei, s4, eo, fi]
    w1 = nc.dram_tensor("w1", [128, NS4, ET, 128], BF16, kind="ExternalInput").ap()
    b1 = nc.dram_tensor("b1", [FE], F32, kind="ExternalInput").ap()
    w2 = nc.dram_tensor("w2", [FE, E], BF16, kind="ExternalInput").ap()
    b2 = nc.dram_tensor("b2", [E], BF16, kind="ExternalInput").ap()
    out = nc.dram_tensor("out", [QR, E], F32, kind="ExternalOutput").ap()

    with tile.TileContext(nc, pool_alloc_mode="queue") as tc:
        consts = tc.alloc_tile_pool(name="consts", bufs=1)
        small = tc.alloc_tile_pool(name="small", bufs=6)

        ident = consts.tile([128, 128], F32)
        make_identity(nc, ident)
        identr = consts.tile([128, 128], F32R, tag="identr")
        nc.vector.tensor_copy(identr, ident)
        onesb = consts.tile([128, 256], BF16, tag="onesb")
        nc.vector.memset(onesb, 1.0)
        epst = consts.tile([128, 1], F32)
        nc.vector.memset(epst, EPS)
        sel_sb = consts.tile([128, 128], BF16, tag="sel")
        nc.sync.dma_start(sel_sb, sel)
        ln1g_sb = consts.tile([128, ET], F32, tag="lnp1")
        nc.sync.dma_start(ln1g_sb, ln1g.rearrange("(eo ei) -> ei eo", ei=128))
        ln1b_sb = consts.tile([128, ET], F32, tag="lnp2")
        nc.sync.dma_start(ln1b_sb, ln1b.rearrange("(eo ei) -> ei eo", ei=128))
        ln2g_sb = consts.tile([128, ET], F32, tag="lnp3")
        nc.sync.dma_start(ln2g_sb, ln2g.rearrange("(eo ei) -> ei eo", ei=128))
        ln2b_sb = consts.tile([128, ET], F32, tag="lnp4")
        nc.sync.dma_start(ln2b_sb, ln2b.rearrange("(eo ei) -> ei eo", ei=128))
        b1_sb = consts.tile([128, NS4], F32, tag="b1")
        nc.sync.dma_start(b1_sb, b1.rearrange("(so si) -> si so", si=128))
        wedges = []
        for w in range(2):
            mt = consts.tile([128, 256], BF16, tag=f"mask{w}", name=f"wedge{w}")
            nc.sync.dma_start(mt, masks[w])
            wedges.append(mt)
        rsums = consts.tile([128, 512], F32, tag="rsums")
        nc.vector.memset(rsums, 1.0)  # rows off {0,64} stay 1.0 (benign)

        def layernorm_rows(x_tiles, n_tiles, nrow_tiles):
            """natural-layout LN stats+center+scale for a list of row tiles"""
            for j in range(nrow_tiles):
                xt = x_tiles[j]
                st = small.tile([128, 2, 6], F32, tag="bnst")
                xr = xt.rearrange("p (a b) -> p a b", a=2)
                for sg in range(2):
                    nc.vector.bn_stats(st[:, sg, :], xr[:, sg, :])
                mv = small.tile([128, 2], F32, tag="bnmv")
                nc.vector.bn_aggr(mv, st)
                rstd = small.tile([128, 1], F32, tag="rstd")
                nc.scalar.activation(rstd, mv[:, 1:2], AF.Sqrt, bias=epst)
                nc.vector.reciprocal(rstd, rstd)
                nc.vector.tensor_scalar(
                    n_tiles[j], xt, mv[:, 0:1], rstd,
                    mybir.AluOpType.subtract, mybir.AluOpType.mult,
                )

        # ---------------- long-lived left-stack pools ----------------
        nrmp = tc.alloc_tile_pool(name="nrm", bufs=1)
        w1s = tc.alloc_tile_pool(name="w1s", bufs=2)
        xn2 = tc.alloc_tile_pool(name="xn2", bufs=2)

        # right stack (LIFO top-down): attention state below, early-released
        # weight/Y1T pools above so they pop first (wk after A; wq/wv/y1t
        # mid-stream; pt/vsb/att after the attention stream).
        att_pool = tc.alloc_tile_pool(name="att", bufs=NPR, side="right")
        QT = [att_pool.tile([128, QR], BF16, tag="qt", name=f"QT{i}") for i in range(NPR)]
        KT = [att_pool.tile([128, S], BF16, tag="kt", name=f"KT{i}") for i in range(NPR)]
        vsb_pool = tc.alloc_tile_pool(name="vsb", bufs=1, side="right")
        VSB = vsb_pool.tile([128, NKB, H, DH + 1], BF16, tag="vsb", name="VSB")
        nc.vector.memset(VSB[:, :, :, DH], 1.0)
        ptp = tc.alloc_tile_pool(name="pt", bufs=2, side="right")
        y1t_pool = tc.alloc_tile_pool(name="y1t", bufs=ET, side="right")
        Y1T = [y1t_pool.tile([128, S], BF16, tag="y1t", name=f"Y1T{i}") for i in range(ET)]
        wvp = tc.alloc_tile_pool(name="wvp", bufs=1, side="right")
        wv_all = wvp.tile([128, ET, H, DH], BF16, tag="wva", name="wv_all")
        wqp = tc.alloc_tile_pool(name="wqp", bufs=1, side="right")
        wq_all = wqp.tile([128, ET, H, DH], BF16, tag="wqa", name="wq_all")
        wkp = tc.alloc_tile_pool(name="wkp", bufs=1, side="right")
        wk_all = wkp.tile([128, ET, H, DH], BF16, tag="wka", name="wk_all")

        # =============== Phase A: LN1 -> Y1T; K proj; Q proj ch0 ===============
        with (
            tc.tile_pool(name="xtn", bufs=2) as xtn,
            tc.tile_pool(name="psT", bufs=2, space="PSUM") as psT,
            tc.tile_pool(name="psP1", bufs=2, space="PSUM") as psP1a,
        ):
            for rc in range(4):
                for g in range(2):
                    xts, n1s = [], []
                    for j in range(2):
                        ri = rc * 4 + g * 2 + j
                        xt = xtn.tile([128, E], F32, tag="xt", name=f"xt{j}")
                        nc.sync.dma_start(xt, xin[ri * 128:(ri + 1) * 128, :])
                        xts.append(xt)
                        n1s.append(xtn.tile([128, E], F32R, tag="n1", name=f"n1s{j}"))
                    # weight DMAs issued behind the first x-row loads so LN
                    # starts immediately; wk arrives before the first K proj
                    if rc == 0 and g == 0:
                        nc.sync.dma_start(wk_all, wk)
                    elif rc == 0 and g == 1:
                        nc.sync.dma_start(wq_all, wq)
                    elif rc == 1 and g == 0:
                        nc.sync.dma_start(wv_all, wv)
                    layernorm_rows(xts, n1s, 2)
                    for e in range(ET):
                        ps = psT.tile([128, 256], F32R, tag="pst")
                        for j in range(2):
                            nc.tensor.transpose(
                                ps[:, j * 128:(j + 1) * 128],
                                n1s[j][:, e * 128:(e + 1) * 128], identr)
                        dst = Y1T[e][:, (rc * 2 + g) * 256:(rc * 2 + g + 1) * 256]
                        if e % 2:
                            nc.scalar.activation(
                                dst, ps.bitcast(F32), AF.Identity,
                                bias=ln1b_sb[:, e:e + 1],
                                scale=ln1g_sb[:, e:e + 1])
                        else:
                            nc.vector.tensor_scalar(
                                dst, ps.bitcast(F32),
                                ln1g_sb[:, e:e + 1], ln1b_sb[:, e:e + 1],
                                mybir.AluOpType.mult, mybir.AluOpType.add)
                # K projection for this 512-key chunk, all head pairs
                for pr in range(NPR):
                    ps = psP1a.tile([128, 512], F32, tag="proj")
                    for e in range(ET):
                        nc.tensor.matmul(
                            ps, wk_all[:, e, 2 * pr:2 * pr + 2, :],
                            Y1T[e][:, rc * 512:(rc + 1) * 512],
                            start=(e == 0), stop=(e == ET - 1))
                    nc.vector.tensor_copy(KT[pr][:, rc * 512:(rc + 1) * 512], ps)
                if rc == 0:
                    for pr in range(NPR):
                        ps = psP1a.tile([128, 512], F32, tag="proj")
                        for e in range(ET):
                            nc.tensor.matmul(
                                ps, wq_all[:, e, 2 * pr:2 * pr + 2, :],
                                Y1T[e][:, 0:512],
                                start=(e == 0), stop=(e == ET - 1))
                        nc.vector.tensor_copy(QT[pr][:, 0:512], ps)
        wkp.release()

        oac_pool = tc.alloc_tile_pool(name="oac", bufs=NPR)
        OACC = [oac_pool.tile([128, QR], BF16, tag="oacc", name=f"OACC{i}")
                for i in range(NPR)]
        x2_pool = tc.alloc_tile_pool(name="x2", bufs=1)
        shared = tc.alloc_tile_pool(name="shr", bufs=2, space="PSUM")
        psS = tc.alloc_tile_pool(name="psS", bufs=2, space="PSUM")
        psO = tc.alloc_tile_pool(name="psO", bufs=2, space="PSUM")

        pools = {}    # mid-stream allocated pools (x2/y2t/h1/w1s/xn2/wop)
        wo_sb_box = {}
        X2 = {}       # chunk -> [128, 4, E] f32 tile (lazy)
        Y2T = {}      # chunk -> [e][128, CH] bf16 (lazy)
        h1t = {0: {}, 1: {}}   # chunk -> s4 -> [128, CH] bf16 (lazy)

        def issue_avs(av):
            ops, pr, kb, q0, pt, first, last = av
            N = CH - q0
            for hh in range(2):
                nc.tensor.matmul(
                    ops[hh][0:DH + 1, q0:CH],
                    VSB[:, kb, 2 * pr + hh, :],
                    pt[:, hh * 512:hh * 512 + N],
                    start=first, stop=last, skip_group_check=True)

        def issue_epilogue(ep):
            ops, pr, ch = ep
            for hh in range(2):
                eng = nc.scalar.copy if hh else nc.vector.tensor_copy
                eng(rsums[64 * hh:64 * hh + 1, :], ops[hh][DH:DH + 1, :])
            for hh in range(2):
                nc.vector.tensor_copy(
                    OACC[pr][hh * 64:(hh + 1) * 64, ch * CH:(ch + 1) * CH],
                    ops[hh][0:DH, :])
            rcp = nrmp.tile([128, 512], F32, tag="rcp")
            nc.vector.reciprocal(rcp, rsums)
            return (rcp, pr, ch)

        def issue_norm(nm, shared_bc=False):
            rcp, pr, ch = nm
            rcpb = nrmp.tile([128, 512], BF16, tag="rcpb")
            nc.vector.tensor_copy(rcpb, rcp)
            if shared_bc:
                bc = shared.tile([128, 512], F32, tag="proj")
                bcv = bc
            else:
                bc = psS.tile([128, 1024], F32, tag="sc")
                bcv = bc[:, 0:512]
            nc.tensor.matmul(
                bcv, sel_sb[0:65, :], rcpb[0:65, :],
                start=True, stop=True)
            bcs = nrmp.tile([128, 512], BF16, tag="bcs")
            nc.vector.tensor_copy(bcs, bcv)
            nc.vector.tensor_mul(
                OACC[pr][:, ch * CH:(ch + 1) * CH],
                OACC[pr][:, ch * CH:(ch + 1) * CH], bcs)

        # ---- filler blocks ----
        def v_block(kb, half):
            def go():
                ps = shared.tile([128, 512], F32, tag="proj")
                for e in range(ET):
                    nc.tensor.matmul(
                        ps, Y1T[e][:, kb * 128:(kb + 1) * 128],
                        wv_all[:, e, 8 * half:8 * half + 8, :],
                        start=(e == 0), stop=(e == ET - 1))
                nc.vector.tensor_copy(
                    VSB[:, kb, 8 * half:8 * half + 8, 0:DH],
                    ps.rearrange("p (h d) -> p h d", h=8))
            return go

        def q1_block(pr):
            def go():
                ps = shared.tile([128, 512], F32, tag="proj")
                for e in range(ET):
                    nc.tensor.matmul(
                        ps, wq_all[:, e, 2 * pr:2 * pr + 2, :],
                        Y1T[e][:, 512:1024],
                        start=(e == 0), stop=(e == ET - 1))
                nc.vector.tensor_copy(QT[pr][:, 512:1024], ps)
            return go

        def release_block():
            def go():
                wqp.release()
                wvp.release()
                y1t_pool.release()
            return go

        def alloc_pools_block():
            def go():
                pools["y2t"] = tc.alloc_tile_pool(name="y2t", bufs=ET)
                pools["h1"] = tc.alloc_tile_pool(name="h1", bufs=NS4)
            return go

        def wo_load_block():
            def go():
                wop = tc.alloc_tile_pool(name="wop", bufs=1)
                pools["wop"] = wop
                wo_sb = wop.tile([128, ET, E], BF16, tag="wo")
                nc.sync.dma_start(
                    wo_sb, wo.rearrange("(po pi) o -> pi po o", pi=128))
                wo_sb_box[0] = wo_sb
                bo_sb = wop.tile([1, E], BF16, tag="bo")
                nc.sync.dma_start(bo_sb, bo[None, :])
                wo_sb_box["bo"] = bo_sb
                b2_sb = wop.tile([1, E], BF16, tag="b2")
                nc.sync.dma_start(b2_sb, b2[None, :])
                wo_sb_box["b2"] = b2_sb
            return go

        def x2_init_block(c, j):
            def go():
                if c not in X2:
                    X2[c] = x2_pool.tile([128, 4, E], F32, tag="x2",
                                         name=f"X2_{c}")
                qt = c * 4 + j
                nc.sync.dma_start(
                    X2[c][:, j, :], xin[qt * 128:(qt + 1) * 128, :])
            return go

        def p3_block(c, qt):
            def go():
                wo_sb = wo_sb_box[0]
                for eh in range(2):
                    ps = shared.tile([128, 512], F32, tag="proj")
                    for pr in range(NPR):
                        nc.tensor.matmul(
                            ps, OACC[pr][:, qt * 128:(qt + 1) * 128],
                            wo_sb[:, pr, eh * 512:(eh + 1) * 512],
                            start=(pr == 0), stop=False)
                    nc.tensor.matmul(
                        ps, onesb[0:1, 0:128],
                        wo_sb_box["bo"][0:1, eh * 512:(eh + 1) * 512],
                        start=False, stop=True)
                    nc.vector.tensor_add(
                        X2[c][:, qt % 4, eh * 512:(eh + 1) * 512],
                        X2[c][:, qt % 4, eh * 512:(eh + 1) * 512], ps)
            return go

        def ln2_block(c, j2):
            def go():
                if c not in Y2T:
                    Y2T[c] = [pools["y2t"].tile([128, CH], BF16, tag="y2t",
                                            name=f"Y2T{c}_{i}")
                              for i in range(ET)]
                x2s = [X2[c][:, j2 * 2 + j, :] for j in range(2)]
                n2s = [xn2.tile([128, E], F32R, tag="n2", name=f"n2s{j}")
                       for j in range(2)]
                layernorm_rows(x2s, n2s, 2)
                for e in range(ET):
                    ps = shared.tile([128, 512], F32R, tag="proj")
                    for j in range(2):
                        nc.tensor.transpose(
                            ps[:, j * 128:(j + 1) * 128],
                            n2s[j][:, e * 128:(e + 1) * 128], identr)
                    if e % 2:
                        nc.scalar.activation(
                            Y2T[c][e][:, j2 * 256:(j2 + 1) * 256],
                            ps[:, 0:256].bitcast(F32), AF.Identity,
                            bias=ln2b_sb[:, e:e + 1],
                            scale=ln2g_sb[:, e:e + 1])
                    else:
                        nc.vector.tensor_scalar(
                            Y2T[c][e][:, j2 * 256:(j2 + 1) * 256],
                            ps[:, 0:256].bitcast(F32),
                            ln2g_sb[:, e:e + 1], ln2b_sb[:, e:e + 1],
                            mybir.AluOpType.mult, mybir.AluOpType.add)
            return go

        def ffn1_block(c, g, pool=None):
            # processes s4 pair (2g, 2g+1) with one double-size weight DMA
            def go():
                w1_sb = w1s.tile([128, 2, ET, 128], BF16, tag="w1")
                nc.sync.dma_start(w1_sb, w1[:, 2 * g:2 * g + 2])
                for k in range(2):
                    s4 = 2 * g + k
                    if pool is not None:
                        ps = pool.tile([128, 512], F32, tag="f2")
                    else:
                        ps = shared.tile([128, 512], F32, tag="proj")
                    for e in range(ET):
                        nc.tensor.matmul(
                            ps, w1_sb[:, k, e, :], Y2T[c][e],
                            start=(e == 0), stop=(e == ET - 1))
                    ht = pools["h1"].tile([128, CH], BF16, tag="h1",
                                          name=f"h1_{c}_{s4}")
                    h1t[c][s4] = ht
                    # bias + relu on DVE (ScalarE stays free for softmax exp)
                    nc.vector.tensor_scalar(
                        ht, ps, b1_sb[:, s4:s4 + 1], 0.0,
                        mybir.AluOpType.add, mybir.AluOpType.max)
            return go

        # ---- filler schedule: slot -> list of blocks ----
        ch0_kbs = (0, 1, 2, 3, 8, 9, 10, 11)
        ch1_kbs = (4, 5, 6, 7, 12, 13, 14, 15)
        sched = {}

        def at(slot, blk):
            sched.setdefault(slot, []).append(blk)

        for i, kb in enumerate(ch0_kbs):
            at(i, v_block(kb, 0))              # c0pr0: JIT for its AVs
            at(8 + i, v_block(kb, 1))          # before pr4 (slot 32)
        for pr in range(NPR):                  # Q1 before ch1 (slot 64)
            at(16 + 4 * pr, q1_block(pr))
        for i, kb in enumerate(ch1_kbs):
            at(18 + 4 * i, v_block(kb, 0))     # before c1pr0 diag AVs (~73)
            at(48 + 3 * i, v_block(kb, 1))     # before c1pr4 (slot 128)
        at(70, release_block())                # after last V block @ 69
        at(71, alloc_pools_block())
        at(72, wo_load_block())
        for qt in range(4):
            at(73 + qt, x2_init_block(0, qt))
            at(77 + 2 * qt, p3_block(0, qt))   # after norm(c0pr7) @ slot 69
        at(85, ln2_block(0, 0))
        at(87, ln2_block(0, 1))
        for g in range(NS4 // 2):              # spread over slots 89..190
            at(89 + (g * 101) // 16, ffn1_block(0, g))

        # ---- the attention stream ----
        units = [(0, pr) for pr in range(NPR)] + [(1, pr) for pr in range(NPR)]
        slot = 0
        pend_ep = None
        norm_q = []
        for ch, pr in units:
            visits = _visits(ch)
            ops = [psO.tile([128, 512], F32, tag="ot", name=f"ot{hh}")
                   for hh in range(2)]
            pend_av = None
            nv = len(visits)
            for vi, (kb, qlo) in enumerate(visits):
                q0 = 0 if qlo is None else qlo
                N = CH - q0
                kcol = kb * 128
                wm = wedges[0 if kb < 8 else 1]
                pss = psS.tile([128, 1024], F32, tag="sc")
                for hh in range(2):
                    nc.tensor.matmul(
                        pss[:, hh * 512:hh * 512 + N],
                        KT[pr][hh * 64:(hh + 1) * 64, kcol:kcol + 128],
                        QT[pr][hh * 64:(hh + 1) * 64,
                               ch * CH + q0:(ch + 1) * CH],
                        start=True, stop=True)
                pt = ptp.tile([128, 1024], BF16, tag="pt")
                if N == 512:
                    nc.scalar.activation(pt, pss, AF.Exp, scale=SC)
                else:
                    pt3 = pt.rearrange("p (h c) -> p h c", h=2)
                    ps3 = pss.rearrange("p (h c) -> p h c", h=2)
                    nc.scalar.activation(pt3[:, :, 0:N], ps3[:, :, 0:N],
                                         AF.Exp, scale=SC)
                if qlo is not None:
                    pt3 = pt.rearrange("p (h c) -> p h c", h=2)
                    nc.vector.tensor_mul(
                        pt3[:, :, 0:128], pt3[:, :, 0:128],
                        wm.rearrange("p (a b) -> p a b", a=2))
                if vi == 0 and pend_ep is not None:
                    norm_q.append(issue_epilogue(pend_ep))
                    pend_ep = None
                if vi == 5 and norm_q:
                    issue_norm(norm_q.pop(0))
                for blk in sched.get(slot, []):
                    blk()
                if pend_av is not None:
                    issue_avs(pend_av)
                pend_av = (ops, pr, kb, q0, pt, vi == 0, vi == nv - 1)
                slot += 1
            issue_avs(pend_av)
            pend_ep = (ops, pr, ch)
        ptp.release()
        vsb_pool.release()
        att_pool.release()
        norm_q.append(issue_epilogue(pend_ep))
        psO.release()
        psS.release()
        for nm in norm_q:
            issue_norm(nm, shared_bc=True)

        # =============== Phase D: FFN2(0) | P3/LN2(1) interleaved ===========
        # w2 fully resident (both halves); X2(0) copied aside so the X2
        # buffer can rotate to chunk 1 while FFN2(0) is still consuming it.
        psF = tc.alloc_tile_pool(name="psF", bufs=4, space="PSUM")
        w2s = tc.alloc_tile_pool(name="w2s", bufs=1)
        otp = tc.alloc_tile_pool(name="otp", bufs=2)
        xsp = tc.alloc_tile_pool(name="xsp", bufs=1)
        w2all = w2s.tile([128, NS4, E], BF16, tag="w2a", name="w2all")
        w2r = w2.rearrange("(so si) e -> si so e", si=128)
        nc.sync.dma_start(w2all[:, 0:2, :], w2r[:, 0:2, :])
        nc.sync.dma_start(w2all[:, 2:4, :], w2r[:, 2:4, :])
        X2S = xsp.tile([128, 4, E], F32, tag="x2s", name="X2S")
        nc.vector.tensor_copy(X2S, X2[0])
        x2_init_block(1, 0)()
        for g in range(2, 16):
            nc.sync.dma_start(w2all[:, 2 * g:2 * (g + 1), :],
                              w2r[:, 2 * g:2 * (g + 1), :])
        for j in range(1, 4):
            x2_init_block(1, j)()

        def ffn2_group(c, eh, j, xsrc):
            ps = psF.tile([128, 512], F32, tag="f2")
            for s4 in range(NS4):
                nc.tensor.matmul(
                    ps, h1t[c][s4][:, j * 128:(j + 1) * 128],
                    w2all[:, s4, eh * 512:(eh + 1) * 512],
                    start=(s4 == 0), stop=False)
            nc.tensor.matmul(
                ps, onesb[0:1, 0:128],
                wo_sb_box["b2"][0:1, eh * 512:(eh + 1) * 512],
                start=False, stop=True)
            qt = c * 4 + j
            ot = otp.tile([128, 512], F32, tag="stg")
            nc.vector.tensor_add(ot, ps,
                                 xsrc[:, j, eh * 512:(eh + 1) * 512])
            nc.sync.dma_start(
                out[qt * 128:(qt + 1) * 128, eh * 512:(eh + 1) * 512], ot)

        d_fill = [(eh, j) for eh in range(2) for j in range(4)]
        d_work = ([lambda qt=qt: p3_block(1, qt)() for qt in range(4, 8)]
                  + [lambda: ln2_block(1, 0)(), lambda: ln2_block(1, 1)()])
        for i in range(8):
            eh, j = d_fill[i]
            ffn2_group(0, eh, j, X2S)
            if i < len(d_work):
                d_work[i]()
        for g in range(NS4 // 2):
            ffn1_block(1, g, psF)()
        for eh in range(2):
            for j in range(4):
                ffn2_group(1, eh, j, X2[1])

        xsp.release()
        otp.release()
        w2s.release()
        psF.release()
        shared.release()
        pools["wop"].release()
        pools["h1"].release()
        pools["y2t"].release()
        x2_pool.release()
        oac_pool.release()
        xn2.release()
        w1s.release()
        nrmp.release()
        small.release()
        consts.release()

    nc.compile()
    return nc


def _prep_inputs(inputs):
    import ml_dtypes
    BF = ml_dtypes.bfloat16
    x = np.ascontiguousarray(inputs["x"], dtype=np.float32)
    selm = np.zeros((128, 128), np.float32)
    selm[0, 0:64] = 1.0
    selm[64, 64:128] = 1.0
    shared = {
        "sel": selm.astype(BF),
        "wq": np.ascontiguousarray(
            np.asarray(inputs["Wq"]).reshape(H, ET, 128, DH)
            .transpose(2, 1, 0, 3)).astype(BF),
        "wk": np.ascontiguousarray(
            np.asarray(inputs["Wk"]).reshape(H, ET, 128, DH)
            .transpose(2, 1, 0, 3)).astype(BF),
        "wv": np.ascontiguousarray(
            np.asarray(inputs["Wv"]).reshape(H, ET, 128, DH)
            .transpose(2, 1, 0, 3)).astype(BF),
        "wo": np.ascontiguousarray(inputs["Wo"]).astype(BF),
        "bo": np.ascontiguousarray(inputs["bo"]).astype(BF),
        "ln1g": np.ascontiguousarray(inputs["ln1_g"], np.float32),
        "ln1b": np.ascontiguousarray(inputs["ln1_b"], np.float32),
        "ln2g": np.ascontiguousarray(inputs["ln2_g"], np.float32),
        "ln2b": np.ascontiguousarray(inputs["ln2_b"], np.float32),
        "w1": np.ascontiguousarray(
            np.asarray(inputs["W1"]).reshape(ET, 128, NS4, 128)
            .transpose(1, 2, 0, 3)).astype(BF),
        "b1": np.ascontiguousarray(inputs["b1"], np.float32),
        "w2": np.ascontiguousarray(inputs["W2"]).astype(BF),
        "b2": np.ascontiguousarray(inputs["b2"]).astype(BF),
    }
    in_maps = []
    for c in range(8):
        b, p = c // 2, c % 2
        perm = np.concatenate([np.arange(p, S, 2), np.arange(1 - p, S, 2)])
        kk = np.arange(128)[:, None]
        qq = np.arange(128)[None, :]
        m = np.zeros((2, 128, 128), np.float32)
        m[0] = (qq >= kk).astype(np.float32)          # own-parity blocks
        if p == 0:
            m[1] = (qq > kk).astype(np.float32)       # other-parity, even core
        else:
            m[1] = (qq >= kk).astype(np.float32)      # other-parity, odd core
        m2 = np.concatenate([m, m], axis=2)           # [2,128,256]: wedge doubled
        im = dict(shared)
        im["xin"] = np.ascontiguousarray(x[b][perm])
        im["masks"] = m2.astype(BF)
        in_maps.append(im)
    return in_maps


def _get_prog():
    global _PROG
    if _PROG is None:
        _PROG = _build()
    return _PROG


def run(inputs, trace=False):
    from concourse.bass_utils import run_bass_kernel_spmd

    nc = _get_prog()
    in_maps = _prep_inputs(inputs)
    kw = {}
    if trace:
        import sys, types
        try:
            from antenv.axon_hooks import get_axon_ntff_profile_hook  # noqa
        except ImportError:
            from trn_agent_boot.trn_boot import _ntff_profile_via_ctypes
            hook = _ntff_profile_via_ctypes("/opt/axon/libaxon_pjrt.so")
            mod = types.ModuleType("antenv.axon_hooks")
            mod.get_axon_ntff_profile_hook = lambda: hook
            sys.modules["antenv.axon_hooks"] = mod
        kw["trace"] = True
    res = run_bass_kernel_spmd(nc, in_maps, core_ids=list(range(8)), **kw)
    x = inputs["x"]
    outp = np.empty((B, S, E), np.float32)
    for c in range(8):
        b, p = c // 2, c % 2
        outp[b, p::2, :] = res.results[c]["out"]
    return outp, res


def kernel(**inputs):
    outp, _ = run(inputs)
    return outp


# revision 33
# speedup vs baseline: 1.0062x; 1.0027x over previous
"""Dense transformer block (B=4,S=2048,E=1024,H=16) on 8 trn2 cores.

Sharding: 2 cores per batch sequence; core parity p takes rows p, p+2, ...
(stride-2 interleave) as its query rows -- this balances causal-attention
work exactly across cores.  Each core's x input is row-permuted to
[q rows (local order), other-parity rows] so every SBUF/DRAM offset in the
SPMD program is compile-time constant; causality is enforced with per-core
0/1 mask tensors (pure data).

v2 -- one long PE-dense software pipeline:
  A: per 512-row chunk: LN1 -> transpose -> K projection (Q proj for chunk0).
  Stream: attention runs pr-granular (one head pair at a time, 8 units per
     q-chunk).  Attention is ScalarE(exp)-bound, so independent PE work is
     issued between visits from an explicit slot schedule: V projection,
     Q proj chunk1, then out-proj + LN2 + FFN1 of q-chunk 0.
  D: FFN2(ch0); out-proj/LN2/FFN1/FFN2 of q-chunk 1 (PE-dense tail).
PSUM: psS (scores, 2x[128,1024]) 4 banks + psO (AV accum, 2x[128,512])
2 banks + shared projection accumulator (2x[128,512]) 2 banks = 8.
"""

import numpy as np

B, S, E, H, DH = 4, 2048, 1024, 16, 64
EPS = 1e-5
QR = S // 2          # q rows per core
CH = 512             # q-chunk (matmul free dim)
NCH = QR // CH       # 2 chunks
NKB = S // 128       # 16 key blocks
ET = E // 128        # 8 E tiles
NPR = H // 2         # 8 head pairs
FE = 4 * E           # ffn hidden
NS4 = FE // 128      # 32 ffn hidden slices
SC = 1.0 / np.sqrt(DH)

_PROG = None


def _visits(ch):
    """per q-chunk: list of (key_block, qlo or None) in accumulation order"""
    if ch == 0:
        return [(kb, 128 * (kb % 8)) for kb in (0, 1, 2, 3, 8, 9, 10, 11)]
    full = [(kb, None) for kb in (0, 1, 2, 3, 8, 9, 10, 11)]
    diag = [(kb, 128 * ((kb - 4) if kb < 8 else (kb - 12)))
            for kb in (4, 5, 6, 7, 12, 13, 14, 15)]
    return full + diag


def _build():
    import concourse.bacc as bacc
    import concourse.tile as tile
    from concourse import mybir
    from concourse.masks import make_identity

    F32 = mybir.dt.float32
    F32R = mybir.dt.float32r
    BF16 = mybir.dt.bfloat16
    AF = mybir.ActivationFunctionType

    nc = bacc.Bacc("TRN2", target_bir_lowering=False, debug=False, num_devices=8)

    xin = nc.dram_tensor("xin", [S, E], F32, kind="ExternalInput").ap()
    masks = nc.dram_tensor("masks", [2, 128, 256], BF16, kind="ExternalInput").ap()
    sel = nc.dram_tensor("sel", [128, 128], BF16, kind="ExternalInput").ap()
    # pre-transposed host-side: [ei, eo, h, d]
    wq = nc.dram_tensor("wq", [128, ET, H, DH], BF16, kind="ExternalInput").ap()
    wk = nc.dram_tensor("wk", [128, ET, H, DH], BF16, kind="ExternalInput").ap()
    wv = nc.dram_tensor("wv", [128, ET, H, DH], BF16, kind="ExternalInput").ap()
    wo = nc.dram_tensor("wo", [E, E], BF16, kind="ExternalInput").ap()
    bo = nc.dram_tensor("bo", [E], BF16, kind="ExternalInput").ap()
    ln1g = nc.dram_tensor("ln1g", [E], F32, kind="ExternalInput").ap()
    ln1b = nc.dram_tensor("ln1b", [E], F32, kind="ExternalInput").ap()
    ln2g = nc.dram_tensor("ln2g", [E], F32, kind="ExternalInput").ap()
    ln2b = nc.dram_tensor("ln2b", [E], F32, kind="ExternalInput").ap()
    # pre-transposed host-side: [ei, s4, eo, fi]
    w1 = nc.dram_tensor("w1", [128, NS4, ET, 128], BF16, kind="ExternalInput").ap()
    b1 = nc.dram_tensor("b1", [FE], F32, kind="ExternalInput").ap()
    w2 = nc.dram_tensor("w2", [FE, E], BF16, kind="ExternalInput").ap()
    b2 = nc.dram_tensor("b2", [E], BF16, kind="ExternalInput").ap()
    out = nc.dram_tensor("out", [QR, E], F32, kind="ExternalOutput").ap()

    with tile.TileContext(nc, pool_alloc_mode="queue") as tc:
        consts = tc.alloc_tile_pool(name="consts", bufs=1)
        small = tc.alloc_tile_pool(name="small", bufs=6)

        ident = consts.tile([128, 128], F32)
        make_identity(nc, ident)
        identr = consts.tile([128, 128], F32R, tag="identr")
        nc.vector.tensor_copy(identr, ident)
        onesb = consts.tile([128, 256], BF16, tag="onesb")
        nc.vector.memset(onesb, 1.0)
        epst = consts.tile([128, 1], F32)
        nc.vector.memset(epst, EPS)
        sel_sb = consts.tile([128, 128], BF16, tag="sel")
        nc.sync.dma_start(sel_sb, sel)
        ln1g_sb = consts.tile([128, ET], F32, tag="lnp1")
        nc.sync.dma_start(ln1g_sb, ln1g.rearrange("(eo ei) -> ei eo", ei=128))
        ln1b_sb = consts.tile([128, ET], F32, tag="lnp2")
        nc.sync.dma_start(ln1b_sb, ln1b.rearrange("(eo ei) -> ei eo", ei=128))
        ln2g_sb = consts.tile([128, ET], F32, tag="lnp3")
        nc.sync.dma_start(ln2g_sb, ln2g.rearrange("(eo ei) -> ei eo", ei=128))
        ln2b_sb = consts.tile([128, ET], F32, tag="lnp4")
        nc.sync.dma_start(ln2b_sb, ln2b.rearrange("(eo ei) -> ei eo", ei=128))
        b1_sb = consts.tile([128, NS4], F32, tag="b1")
        nc.sync.dma_start(b1_sb, b1.rearrange("(so si) -> si so", si=128))
        wedges = []
        for w in range(2):
            mt = consts.tile([128, 256], BF16, tag=f"mask{w}", name=f"wedge{w}")
            nc.sync.dma_start(mt, masks[w])
            wedges.append(mt)
        rsums = consts.tile([128, 512], F32, tag="rsums")
        nc.vector.memset(rsums, 1.0)  # rows off {0,64} stay 1.0 (benign)

        def layernorm_rows(x_tiles, n_tiles, nrow_tiles):
            """natural-layout LN stats+center+scale for a list of row tiles"""
            for j in range(nrow_tiles):
                xt = x_tiles[j]
                st = small.tile([128, 2, 6], F32, tag="bnst")
                xr = xt.rearrange("p (a b) -> p a b", a=2)
                for sg in range(2):
                    nc.vector.bn_stats(st[:, sg, :], xr[:, sg, :])
                mv = small.tile([128, 2], F32, tag="bnmv")
                nc.vector.bn_aggr(mv, st)
                rstd = small.tile([128, 1], F32, tag="rstd")
                nc.scalar.activation(rstd, mv[:, 1:2], AF.Sqrt, bias=epst)
                nc.vector.reciprocal(rstd, rstd)
                nc.vector.tensor_scalar(
                    n_tiles[j], xt, mv[:, 0:1], rstd,
                    mybir.AluOpType.subtract, mybir.AluOpType.mult,
                )

        # ---------------- long-lived left-stack pools ----------------
        nrmp = tc.alloc_tile_pool(name="nrm", bufs=1)
        w1s = tc.alloc_tile_pool(name="w1s", bufs=2)
        xn2 = tc.alloc_tile_pool(name="xn2", bufs=2)

        # right stack (LIFO top-down): attention state below, early-released
        # weight/Y1T pools above so they pop first (wk after A; wq/wv/y1t
        # mid-stream; pt/vsb/att after the attention stream).
        att_pool = tc.alloc_tile_pool(name="att", bufs=NPR, side="right")
        QT = [att_pool.tile([128, QR], BF16, tag="qt", name=f"QT{i}") for i in range(NPR)]
        KT = [att_pool.tile([128, S], BF16, tag="kt", name=f"KT{i}") for i in range(NPR)]
        vsb_pool = tc.alloc_tile_pool(name="vsb", bufs=1, side="right")
        VSB = vsb_pool.tile([128, NKB, H, DH + 1], BF16, tag="vsb", name="VSB")
        nc.vector.memset(VSB[:, :, :, DH], 1.0)
        ptp = tc.alloc_tile_pool(name="pt", bufs=2, side="right")
        y1t_pool = tc.alloc_tile_pool(name="y1t", bufs=ET, side="right")
        Y1T = [y1t_pool.tile([128, S], BF16, tag="y1t", name=f"Y1T{i}") for i in range(ET)]
        wvp = tc.alloc_tile_pool(name="wvp", bufs=1, side="right")
        wv_all = wvp.tile([128, ET, H, DH], BF16, tag="wva", name="wv_all")
        wqp = tc.alloc_tile_pool(name="wqp", bufs=1, side="right")
        wq_all = wqp.tile([128, ET, H, DH], BF16, tag="wqa", name="wq_all")
        wkp = tc.alloc_tile_pool(name="wkp", bufs=1, side="right")
        wk_all = wkp.tile([128, ET, H, DH], BF16, tag="wka", name="wk_all")

        # =============== Phase A: LN1 -> Y1T; K proj; Q proj ch0 ===============
        with (
            tc.tile_pool(name="xtn", bufs=2) as xtn,
            tc.tile_pool(name="psT", bufs=2, space="PSUM") as psT,
            tc.tile_pool(name="psP1", bufs=2, space="PSUM") as psP1a,
        ):
            for rc in range(4):
                for g in range(2):
                    xts, n1s = [], []
                    for j in range(2):
                        ri = rc * 4 + g * 2 + j
                        xt = xtn.tile([128, E], F32, tag="xt", name=f"xt{j}")
                        nc.sync.dma_start(xt, xin[ri * 128:(ri + 1) * 128, :])
                        xts.append(xt)
                        n1s.append(xtn.tile([128, E], F32R, tag="n1", name=f"n1s{j}"))
                    # weight DMAs issued behind the first x-row loads so LN
                    # starts immediately; wk arrives before the first K proj
                    if rc == 0 and g == 0:
                        nc.sync.dma_start(wk_all, wk)
                    elif rc == 0 and g == 1:
                        nc.sync.dma_start(wq_all, wq)
                    elif rc == 1 and g == 0:
                        nc.sync.dma_start(wv_all, wv)
                    layernorm_rows(xts, n1s, 2)
                    for e in range(ET):
                        ps = psT.tile([128, 256], F32R, tag="pst")
                        for j in range(2):
                            nc.tensor.transpose(
                                ps[:, j * 128:(j + 1) * 128],
                                n1s[j][:, e * 128:(e + 1) * 128], identr)
                        dst = Y1T[e][:, (rc * 2 + g) * 256:(rc * 2 + g + 1) * 256]
                        if e % 2:
                            nc.scalar.activation(
                                dst, ps.bitcast(F32), AF.Identity,
                                bias=ln1b_sb[:, e:e + 1],
                                scale=ln1g_sb[:, e:e + 1])
                        else:
                            nc.vector.tensor_scalar(
                                dst, ps.bitcast(F32),
                                ln1g_sb[:, e:e + 1], ln1b_sb[:, e:e + 1],
                                mybir.AluOpType.mult, mybir.AluOpType.add)
                # K projection for this 512-key chunk, all head pairs
                for pr in range(NPR):
                    ps = psP1a.tile([128, 512], F32, tag="proj")
                    for e in range(ET):
                        nc.tensor.matmul(
                            ps, wk_all[:, e, 2 * pr:2 * pr + 2, :],
                            Y1T[e][:, rc * 512:(rc + 1) * 512],
                            start=(e == 0), stop=(e == ET - 1))
                    nc.vector.tensor_copy(KT[pr][:, rc * 512:(rc + 1) * 512], ps)
                if rc == 0:
                    for pr in range(NPR):
                        ps = psP1a.tile([128, 512], F32, tag="proj")
                        for e in range(ET):
                            nc.tensor.matmul(
                                ps, wq_all[:, e, 2 * pr:2 * pr + 2, :],
                                Y1T[e][:, 0:512],
                                start=(e == 0), stop=(e == ET - 1))
                        nc.vector.tensor_copy(QT[pr][:, 0:512], ps)
        wkp.release()

        oac_pool = tc.alloc_tile_pool(name="oac", bufs=NPR)
        OACC = [oac_pool.tile([128, QR], BF16, tag="oacc", name=f"OACC{i}")
                for i in range(NPR)]
        x2_pool = tc.alloc_tile_pool(name="x2", bufs=1)
        shared = tc.alloc_tile_pool(name="shr", bufs=2, space="PSUM")
        psS = tc.alloc_tile_pool(name="psS", bufs=2, space="PSUM")
        psO = tc.alloc_tile_pool(name="psO", bufs=2, space="PSUM")

        pools = {}    # mid-stream allocated pools (x2/y2t/h1/w1s/xn2/wop)
        wo_sb_box = {}
        X2 = {}       # chunk -> [128, 4, E] f32 tile (lazy)
        Y2T = {}      # chunk -> [e][128, CH] bf16 (lazy)
        h1t = {0: {}, 1: {}}   # chunk -> s4 -> [128, CH] bf16 (lazy)

        def issue_avs(av):
            ops, pr, kb, q0, pt, first, last = av
            N = CH - q0
            for hh in range(2):
                nc.tensor.matmul(
                    ops[hh][0:DH + 1, q0:CH],
                    VSB[:, kb, 2 * pr + hh, :],
                    pt[:, hh * 512:hh * 512 + N],
                    start=first, stop=last, skip_group_check=True)

        def issue_epilogue(ep):
            ops, pr, ch = ep
            for hh in range(2):
                eng = nc.scalar.copy if hh else nc.vector.tensor_copy
                eng(rsums[64 * hh:64 * hh + 1, :], ops[hh][DH:DH + 1, :])
            for hh in range(2):
                nc.vector.tensor_copy(
                    OACC[pr][hh * 64:(hh + 1) * 64, ch * CH:(ch + 1) * CH],
                    ops[hh][0:DH, :])
            rcp = nrmp.tile([128, 512], F32, tag="rcp")
            nc.vector.reciprocal(rcp, rsums)
            return (rcp, pr, ch)

        def issue_norm(nm, shared_bc=False):
            rcp, pr, ch = nm
            rcpb = nrmp.tile([128, 512], BF16, tag="rcpb")
            nc.vector.tensor_copy(rcpb, rcp)
            if shared_bc:
                bc = shared.tile([128, 512], F32, tag="proj")
                bcv = bc
            else:
                bc = psS.tile([128, 1024], F32, tag="sc")
                bcv = bc[:, 0:512]
            nc.tensor.matmul(
                bcv, sel_sb[0:65, :], rcpb[0:65, :],
                start=True, stop=True)
            bcs = nrmp.tile([128, 512], BF16, tag="bcs")
            nc.vector.tensor_copy(bcs, bcv)
            nc.vector.tensor_mul(
                OACC[pr][:, ch * CH:(ch + 1) * CH],
                OACC[pr][:, ch * CH:(ch + 1) * CH], bcs)

        # ---- filler blocks ----
        def v_block(kb, half):
            def go():
                ps = shared.tile([128, 512], F32, tag="proj")
                for e in range(ET):
                    nc.tensor.matmul(
                        ps, Y1T[e][:, kb * 128:(kb + 1) * 128],
                        wv_all[:, e, 8 * half:8 * half + 8, :],
                        start=(e == 0), stop=(e == ET - 1))
                nc.vector.tensor_copy(
                    VSB[:, kb, 8 * half:8 * half + 8, 0:DH],
                    ps.rearrange("p (h d) -> p h d", h=8))
            return go

        def q1_block(pr):
            def go():
                ps = shared.tile([128, 512], F32, tag="proj")
                for e in range(ET):
                    nc.tensor.matmul(
                        ps, wq_all[:, e, 2 * pr:2 * pr + 2, :],
                        Y1T[e][:, 512:1024],
                        start=(e == 0), stop=(e == ET - 1))
                nc.vector.tensor_copy(QT[pr][:, 512:1024], ps)
            return go

        def release_block():
            def go():
                wqp.release()
                wvp.release()
                y1t_pool.release()
            return go

        def alloc_pools_block():
            def go():
                pools["y2t"] = tc.alloc_tile_pool(name="y2t", bufs=ET)
                pools["h1"] = tc.alloc_tile_pool(name="h1", bufs=NS4)
            return go

        def wo_load_block():
            def go():
                wop = tc.alloc_tile_pool(name="wop", bufs=1)
                pools["wop"] = wop
                wo_sb = wop.tile([128, ET, E], BF16, tag="wo")
                nc.sync.dma_start(
                    wo_sb, wo.rearrange("(po pi) o -> pi po o", pi=128))
                wo_sb_box[0] = wo_sb
                bo_sb = wop.tile([1, E], BF16, tag="bo")
                nc.sync.dma_start(bo_sb, bo[None, :])
                wo_sb_box["bo"] = bo_sb
                b2_sb = wop.tile([1, E], BF16, tag="b2")
                nc.sync.dma_start(b2_sb, b2[None, :])
                wo_sb_box["b2"] = b2_sb
            return go

        def x2_init_block(c, j):
            def go():
                if c not in X2:
                    X2[c] = x2_pool.tile([128, 4, E], F32, tag="x2",
                                         name=f"X2_{c}")
                qt = c * 4 + j
                nc.sync.dma_start(
                    X2[c][:, j, :], xin[qt * 128:(qt + 1) * 128, :])
            return go

        def p3_block(c, qt):
            def go():
                wo_sb = wo_sb_box[0]
                for eh in range(2):
                    ps = shared.tile([128, 512], F32, tag="proj")
                    for pr in range(NPR):
                        nc.tensor.matmul(
                            ps, OACC[pr][:, qt * 128:(qt + 1) * 128],
                            wo_sb[:, pr, eh * 512:(eh + 1) * 512],
                            start=(pr == 0), stop=False)
                    nc.tensor.matmul(
                        ps, onesb[0:1, 0:128],
                        wo_sb_box["bo"][0:1, eh * 512:(eh + 1) * 512],
                        start=False, stop=True)
                    nc.vector.tensor_add(
                        X2[c][:, qt % 4, eh * 512:(eh + 1) * 512],
                        X2[c][:, qt % 4, eh * 512:(eh + 1) * 512], ps)
            return go

        def ln2_block(c, j2):
            def go():
                if c not in Y2T:
                    Y2T[c] = [pools["y2t"].tile([128, CH], BF16, tag="y2t",
                                            name=f"Y2T{c}_{i}")
                              for i in range(ET)]
                x2s = [X2[c][:, j2 * 2 + j, :] for j in range(2)]
                n2s = [xn2.tile([128, E], F32R, tag="n2", name=f"n2s{j}")
                       for j in range(2)]
                layernorm_rows(x2s, n2s, 2)
                for e in range(ET):
                    ps = shared.tile([128, 512], F32R, tag="proj")
                    for j in range(2):
                        nc.tensor.transpose(
                            ps[:, j * 128:(j + 1) * 128],
                            n2s[j][:, e * 128:(e + 1) * 128], identr)
                    if e % 2:
                        nc.scalar.activation(
                            Y2T[c][e][:, j2 * 256:(j2 + 1) * 256],
                            ps[:, 0:256].bitcast(F32), AF.Identity,
                            bias=ln2b_sb[:, e:e + 1],
                            scale=ln2g_sb[:, e:e + 1])
                    else:
                        nc.vector.tensor_scalar(
                            Y2T[c][e][:, j2 * 256:(j2 + 1) * 256],
                            ps[:, 0:256].bitcast(F32),
                            ln2g_sb[:, e:e + 1], ln2b_sb[:, e:e + 1],
                            mybir.AluOpType.mult, mybir.AluOpType.add)
            return go

        def ffn1_block(c, g, pool=None):
            # processes s4 pair (2g, 2g+1) with one double-size weight DMA
            def go():
                w1_sb = w1s.tile([128, 2, ET, 128], BF16, tag="w1")
                nc.sync.dma_start(w1_sb, w1[:, 2 * g:2 * g + 2])
                for k in range(2):
                    s4 = 2 * g + k
                    if pool is not None:
                        ps = pool.tile([128, 512], F32, tag="f2")
                    else:
                        ps = shared.tile([128, 512], F32, tag="proj")
                    for e in range(ET):
                        nc.tensor.matmul(
                            ps, w1_sb[:, k, e, :], Y2T[c][e],
                            start=(e == 0), stop=(e == ET - 1))
                    ht = pools["h1"].tile([128, CH], BF16, tag="h1",
                                          name=f"h1_{c}_{s4}")
                    h1t[c][s4] = ht
                    # bias + relu on DVE (ScalarE stays free for softmax exp)
                    nc.vector.tensor_scalar(
                        ht, ps, b1_sb[:, s4:s4 + 1], 0.0,
                        mybir.AluOpType.add, mybir.AluOpType.max)
            return go

        # ---- filler schedule: slot -> list of blocks ----
        ch0_kbs = (0, 1, 2, 3, 8, 9, 10, 11)
        ch1_kbs = (4, 5, 6, 7, 12, 13, 14, 15)
        sched = {}

        def at(slot, blk):
            sched.setdefault(slot, []).append(blk)

        for i, kb in enumerate(ch0_kbs):
            at(i, v_block(kb, 0))              # c0pr0: JIT for its AVs
            at(8 + i, v_block(kb, 1))          # before pr4 (slot 32)
        for pr in range(NPR):                  # Q1 before ch1 (slot 64)
            at(16 + 4 * pr, q1_block(pr))
        for i, kb in enumerate(ch1_kbs):
            at(18 + 4 * i, v_block(kb, 0))     # before c1pr0 diag AVs (~73)
            at(48 + 3 * i, v_block(kb, 1))     # before c1pr4 (slot 128)
        at(70, release_block())                # after last V block @ 69
        at(71, alloc_pools_block())
        at(72, wo_load_block())
        for qt in range(4):
            at(73 + qt, x2_init_block(0, qt))
            at(77 + 2 * qt, p3_block(0, qt))   # after norm(c0pr7) @ slot 69
        at(85, ln2_block(0, 0))
        at(87, ln2_block(0, 1))
        for g in range(NS4 // 2):              # spread over slots 89..190
            at(89 + (g * 101) // 16, ffn1_block(0, g))

        # ---- the attention stream ----
        units = [(0, pr) for pr in range(NPR)] + [(1, pr) for pr in range(NPR)]
        slot = 0
        pend_ep = None
        norm_q = []
        for ch, pr in units:
            visits = _visits(ch)
            ops = [psO.tile([128, 512], F32, tag="ot", name=f"ot{hh}")
                   for hh in range(2)]
            pend_av = None
            nv = len(visits)
            for vi, (kb, qlo) in enumerate(visits):
                q0 = 0 if qlo is None else qlo
                N = CH - q0
                kcol = kb * 128
                wm = wedges[0 if kb < 8 else 1]
                pss = psS.tile([128, 1024], F32, tag="sc")
                for hh in range(2):
                    nc.tensor.matmul(
                        pss[:, hh * 512:hh * 512 + N],
                        KT[pr][hh * 64:(hh + 1) * 64, kcol:kcol + 128],
                        QT[pr][hh * 64:(hh + 1) * 64,
                               ch * CH + q0:(ch + 1) * CH],
                        start=True, stop=True)
                pt = ptp.tile([128, 1024], BF16, tag="pt")
                if N == 512:
                    nc.scalar.activation(pt, pss, AF.Exp, scale=SC)
                else:
                    pt3 = pt.rearrange("p (h c) -> p h c", h=2)
                    ps3 = pss.rearrange("p (h c) -> p h c", h=2)
                    nc.scalar.activation(pt3[:, :, 0:N], ps3[:, :, 0:N],
                                         AF.Exp, scale=SC)
                if qlo is not None:
                    pt3 = pt.rearrange("p (h c) -> p h c", h=2)
                    nc.vector.tensor_mul(
                        pt3[:, :, 0:128], pt3[:, :, 0:128],
                        wm.rearrange("p (a b) -> p a b", a=2))
                if vi == 1 and pend_ep is not None:
                    norm_q.append(issue_epilogue(pend_ep))
                    pend_ep = None
                if vi == 5 and norm_q:
                    issue_norm(norm_q.pop(0))
                if pend_av is not None:
                    issue_avs(pend_av)
                pend_av = (ops, pr, kb, q0, pt, vi == 0, vi == nv - 1)
                for blk in sched.get(slot, []):
                    blk()
                slot += 1
            issue_avs(pend_av)
            pend_ep = (ops, pr, ch)
        ptp.release()
        vsb_pool.release()
        att_pool.release()
        norm_q.append(issue_epilogue(pend_ep))
        psO.release()
        psS.release()
        for nm in norm_q:
            issue_norm(nm, shared_bc=True)

        # =============== Phase D: FFN2(0) | P3/LN2(1) interleaved ===========
        # w2 fully resident (both halves); X2(0) copied aside so the X2
        # buffer can rotate to chunk 1 while FFN2(0) is still consuming it.
        psF = tc.alloc_tile_pool(name="psF", bufs=6, space="PSUM")
        w2s = tc.alloc_tile_pool(name="w2s", bufs=1)
        otp = tc.alloc_tile_pool(name="otp", bufs=2)
        xsp = tc.alloc_tile_pool(name="xsp", bufs=1)
        w2all = w2s.tile([128, NS4, E], BF16, tag="w2a", name="w2all")
        w2r = w2.rearrange("(so si) e -> si so e", si=128)
        nc.sync.dma_start(w2all[:, 0:2, :], w2r[:, 0:2, :])
        nc.sync.dma_start(w2all[:, 2:4, :], w2r[:, 2:4, :])
        X2S = xsp.tile([128, 4, E], F32, tag="x2s", name="X2S")
        nc.vector.tensor_copy(X2S, X2[0])
        x2_init_block(1, 0)()
        for g in range(2, 16):
            nc.sync.dma_start(w2all[:, 2 * g:2 * (g + 1), :],
                              w2r[:, 2 * g:2 * (g + 1), :])
        for j in range(1, 4):
            x2_init_block(1, j)()

        def ffn2_group(c, eh, j, xsrc):
            ps = psF.tile([128, 512], F32, tag="f2")
            for s4 in range(NS4):
                nc.tensor.matmul(
                    ps, h1t[c][s4][:, j * 128:(j + 1) * 128],
                    w2all[:, s4, eh * 512:(eh + 1) * 512],
                    start=(s4 == 0), stop=False)
            nc.tensor.matmul(
                ps, onesb[0:1, 0:128],
                wo_sb_box["b2"][0:1, eh * 512:(eh + 1) * 512],
                start=False, stop=True)
            qt = c * 4 + j
            ot = otp.tile([128, 512], F32, tag="stg")
            nc.vector.tensor_add(ot, ps,
                                 xsrc[:, j, eh * 512:(eh + 1) * 512])
            nc.sync.dma_start(
                out[qt * 128:(qt + 1) * 128, eh * 512:(eh + 1) * 512], ot)

        d_fill = [(eh, j) for eh in range(2) for j in range(4)]
        d_work = ([lambda qt=qt: p3_block(1, qt)() for qt in range(4, 8)]
                  + [lambda: ln2_block(1, 0)(), lambda: ln2_block(1, 1)()])
        for i in range(8):
            eh, j = d_fill[i]
            ffn2_group(0, eh, j, X2S)
            if i < len(d_work):
                d_work[i]()
        for g in range(NS4 // 2):
            ffn1_block(1, g, psF)()
        for eh in range(2):
            for j in range(4):
                ffn2_group(1, eh, j, X2[1])

        xsp.release()
        otp.release()
        w2s.release()
        psF.release()
        shared.release()
        pools["wop"].release()
        pools["h1"].release()
        pools["y2t"].release()
        x2_pool.release()
        oac_pool.release()
        xn2.release()
        w1s.release()
        nrmp.release()
        small.release()
        consts.release()

    nc.compile()
    return nc


def _prep_inputs(inputs):
    import ml_dtypes
    BF = ml_dtypes.bfloat16
    x = np.ascontiguousarray(inputs["x"], dtype=np.float32)
    selm = np.zeros((128, 128), np.float32)
    selm[0, 0:64] = 1.0
    selm[64, 64:128] = 1.0
    shared = {
        "sel": selm.astype(BF),
        "wq": np.ascontiguousarray(
            np.asarray(inputs["Wq"]).reshape(H, ET, 128, DH)
            .transpose(2, 1, 0, 3)).astype(BF),
        "wk": np.ascontiguousarray(
            np.asarray(inputs["Wk"]).reshape(H, ET, 128, DH)
            .transpose(2, 1, 0, 3)).astype(BF),
        "wv": np.ascontiguousarray(
            np.asarray(inputs["Wv"]).reshape(H, ET, 128, DH)
            .transpose(2, 1, 0, 3)).astype(BF),
        "wo": np.ascontiguousarray(inputs["Wo"]).astype(BF),
        "bo": np.ascontiguousarray(inputs["bo"]).astype(BF),
        "ln1g": np.ascontiguousarray(inputs["ln1_g"], np.float32),
        "ln1b": np.ascontiguousarray(inputs["ln1_b"], np.float32),
        "ln2g": np.ascontiguousarray(inputs["ln2_g"], np.float32),
        "ln2b": np.ascontiguousarray(inputs["ln2_b"], np.float32),
        "w1": np.ascontiguousarray(
            np.asarray(inputs["W1"]).reshape(ET, 128, NS4, 128)
            .transpose(1, 2, 0, 3)).astype(BF),
        "b1": np.ascontiguousarray(inputs["b1"], np.float32),
        "w2": np.ascontiguousarray(inputs["W2"]).astype(BF),
        "b2": np.ascontiguousarray(inputs["b2"]).astype(BF),
    }
    in_maps = []
    for c in range(8):
        b, p = c // 2, c % 2
        perm = np.concatenate([np.arange(p, S, 2), np.arange(1 - p, S, 2)])
        kk = np.arange(128)[:, None]
        qq = np.arange(128)[None, :]
        m = np.zeros((2, 128, 128), np.float32)
        m[0] = (qq >= kk).astype(np.float32)          # own-parity blocks
        if p == 0:
            m[1] = (qq > kk).astype(np.float32)       # other-parity, even core
        else:
            m[1] = (qq >= kk).astype(np.float32)      # other-parity, odd core
        m2 = np.concatenate([m, m], axis=2)           # [2,128,256]: wedge doubled
        im = dict(shared)
        im["xin"] = np.ascontiguousarray(x[b][perm])
        im["masks"] = m2.astype(BF)
        in_maps.append(im)
    return in_maps


def _get_prog():
    global _PROG
    if _PROG is None:
        _PROG = _build()
    return _PROG


def run(inputs, trace=False):
    from concourse.bass_utils import run_bass_kernel_spmd

    nc = _get_prog()
    in_maps = _prep_inputs(inputs)
    kw = {}
    if trace:
        import sys, types
        try:
            from antenv.axon_hooks import get_axon_ntff_profile_hook  # noqa
        except ImportError:
            from trn_agent_boot.trn_boot import _ntff_profile_via_ctypes
            hook = _ntff_profile_via_ctypes("/opt/axon/libaxon_pjrt.so")
            mod = types.ModuleType("antenv.axon_hooks")
            mod.get_axon_ntff_profile_hook = lambda: hook
            sys.modules["antenv.axon_hooks"] = mod
        kw["trace"] = True
    res = run_bass_kernel_spmd(nc, in_maps, core_ids=list(range(8)), **kw)
    x = inputs["x"]
    outp = np.empty((B, S, E), np.float32)
    for c in range(8):
        b, p = c // 2, c % 2
        outp[b, p::2, :] = res.results[c]["out"]
    return outp, res


def kernel(**inputs):
    outp, _ = run(inputs)
    return outp
